# revision 2
# baseline (speedup 1.0000x reference)
"""NestedGIN (4-layer GIN + 2-level pooling + MLP head) on 8 Trainium2 NeuronCores.

Strategy:
  - Nodes (and their incident in-edges, i.e. edges grouped by dst) are sharded
    across 8 cores; MLP weights are replicated.
  - Per layer: each core gathers h[src] for its edges from a replicated
    node-major copy of h in HBM (hardware dma_gather, 256B rows), then
    scatter-adds into its node slice via one-hot matmuls on the TensorEngine
    (edges sorted by dst; 128-edge tiles vs 128-node windows; psum窗 flushed
    into a feature-major SBUF accumulator at data-driven dynamic offsets).
  - The GIN MLP runs feature-major ([64, nodes] tiles, weights stationary).
  - The updated slice is transposed back to node-major, exported to HBM and
    AllGather'ed for the next layer's gathers.
  - Final: subgraph pooling (one-hot matmul vs node_to_subgraph) -> AllReduce
    of partial subgraph sums -> graph pooling -> MLP head -> log_softmax.

Host-side numpy does only index/layout prep (sharding, sorting, padding) plus
the layer-0 input-feature reindex x[src] (pure copy, no arithmetic).
"""

import os
import sys

for _p in ("/opt/trn_rl_repo", "/opt/pypackages"):
    if os.path.isdir(_p) and _p not in sys.path:
        sys.path.append(_p)

import numpy as np

import concourse.bass as bass
import concourse.bacc as bacc
import concourse.tile as tile
import concourse.mybir as mybir

F32 = mybir.dt.float32
I32 = mybir.dt.int32
I16 = mybir.dt.int16


class Cfg:
    def __init__(self, N=100000, E=1600000, S=2000, NGRAPH=64, OUT=8,
                 CORES=8, CHUNK=25000):
        self.N, self.E, self.S = N, S and N and E, S  # keep E explicit below
        self.E = E
        self.NGRAPH, self.OUT, self.CORES = NGRAPH, OUT, CORES
        self.H = 64
        self.NPC = N // CORES                      # nodes per core
        assert N % CORES == 0
        self.CHUNK = CHUNK                         # gather-source chunk rows (int16 idx limit)
        assert CHUNK <= 32767
        self.NCH = (N + CHUNK - 1) // CHUNK        # chunks
        assert N % CHUNK == 0
        self.GT = 4                                # edge tiles (128 edges) per group
        self.BG = 8                                # groups per dma_gather batch
        self.W = 160                               # node window per group
        self.NPC_PAD = ((self.NPC + 511) // 512) * 512
        self.NMG = self.NPC_PAD // 512             # MLP node-groups of 512
        self.NT_REAL = (self.NPC + 127) // 128     # node-major tiles (transpose/export)
        self.LAST_ROWS = self.NPC - (self.NT_REAL - 1) * 128
        self.SPAD = 2048                           # padded #subgraphs
        assert S <= self.SPAD
        self.SWIN = 512                            # per-core subgraph window
        self.S_T = self.SPAD // 128
        assert NGRAPH <= 64


# ----------------------------------------------------------------------------
# Host-side prep: shard + sort edges, build tiles/groups, pack device inputs.
# ----------------------------------------------------------------------------

def _prep(inputs, C: Cfg):
    x = np.asarray(inputs["x"], np.float32).reshape(C.N)
    ei = np.asarray(inputs["edge_index"]).astype(np.int64)
    n2s = np.asarray(inputs["node_to_subgraph"]).astype(np.int64)
    s2g = np.asarray(inputs["subgraph_to_graph"]).astype(np.int64)
    src_all, dst_all = ei[0], ei[1]

    # -------- per-core edge groups --------
    per_core = []  # per core: list over chunks of list of groups
    for c in range(C.CORES):
        lo = c * C.NPC
        m = (dst_all >= lo) & (dst_all < lo + C.NPC)
        s, d = src_all[m], dst_all[m]
        ch = s // C.CHUNK
        o = np.lexsort((s, d, ch))
        s, d, ch = s[o], d[o], ch[o]
        chunks = []
        for cv in range(C.NCH):
            sel = ch == cv
            ss, dd = s[sel], d[sel]
            groups = []
            i, n = 0, len(ss)
            cap = C.GT * 128
            while i < n:
                j = min(i + cap, n)
                # keep node span < W
                if dd[j - 1] - dd[i] >= C.W:
                    j = int(np.searchsorted(dd, dd[i] + C.W, side="left"))
                base = int(dd[i])
                gs = ss[i:j]
                gd = dd[i:j] - base
                pad = cap - (j - i)
                if pad:
                    gs = np.concatenate([gs, np.zeros(pad, np.int64) + cv * C.CHUNK])
                    gd = np.concatenate([gd, np.full(pad, -1, np.int64)])
                groups.append((base - lo, gs, gd))
                i = j
            chunks.append(groups)
        per_core.append(chunks)

    # uniform group counts per chunk across cores (SPMD)
    NG = []
    for cv in range(C.NCH):
        mx = max(len(per_core[c][cv]) for c in range(C.CORES))
        mx = ((mx + C.BG - 1) // C.BG) * C.BG
        NG.append(mx)
    NGTOT = sum(NG)
    NBTOT = NGTOT // C.BG
    cap = C.GT * 128

    pad_group = (C.NPC, np.zeros(cap, np.int64), np.full(cap, -1, np.int64))

    in_maps = []
    for c in range(C.CORES):
        lo = c * C.NPC
        dstl = np.empty((128, NGTOT * C.GT), np.float32)
        xsrc = np.zeros((128, NGTOT * C.GT), np.float32)
        flush = np.empty((1, NGTOT), np.int32)
        bcols = C.BG * cap // 16
        idx = np.empty((16, NBTOT * bcols), np.int16)  # replicated to 128 below
        g_i = 0
        for cv in range(C.NCH):
            groups = per_core[c][cv]
            for k in range(NG[cv]):
                off, gs, gd = groups[k] if k < len(groups) else \
                    (pad_group[0], pad_group[1] + cv * C.CHUNK, pad_group[2])
                flush[0, g_i] = off
                dstl[:, g_i * C.GT:(g_i + 1) * C.GT] = \
                    gd.reshape(C.GT, 128).T.astype(np.float32)
                xv = x[gs]
                xv[gd < 0] = 0.0
                xsrc[:, g_i * C.GT:(g_i + 1) * C.GT] = xv.reshape(C.GT, 128).T
                # wrapped int16 idx layout for this group's slot of its batch
                loc = (gs - cv * C.CHUNK).astype(np.int16)
                b = g_i // C.BG
                half = g_i % C.BG
                gcols = cap // 16
                blk = loc.reshape(gcols, 16).T
                idx[:, b * bcols + half * gcols:b * bcols + (half + 1) * gcols] = blk
                g_i += 1
        assert g_i == NGTOT

        own_n2s = n2s[lo:lo + C.NPC]
        sstart = int(min(max(0, own_n2s.min()), C.SPAD - C.SWIN))
        assert own_n2s.max() - sstart < C.SWIN, "subgraph window overflow"
        n2s_lin = np.full(C.NT_REAL * 128, -1, np.float32)
        n2s_lin[:C.NPC] = own_n2s - sstart
        n2s_loc = n2s_lin.reshape(C.NT_REAL, 128).T

        s2g_lin = np.full(C.SPAD, -1, np.float32)
        s2g_lin[:C.S] = s2g
        s2g_loc = s2g_lin.reshape(C.S_T, 128).T

        xown = np.zeros((1, C.NPC_PAD), np.float32)
        xown[0, :C.NPC] = x[lo:lo + C.NPC]

        m = {
            "g_idx": np.tile(idx, (8, 1)), "g_dstl": dstl, "g_xsrc": xsrc, "g_flush": flush,
            "xown": xown, "n2s": n2s_loc.astype(np.float32),
            "s2g": s2g_loc.astype(np.float32),
            "sstart": np.array([[sstart]], np.int32),
            "w1a": np.asarray(inputs["conv1_w1"], np.float32),
            "b1a": np.asarray(inputs["conv1_b1"], np.float32).reshape(C.H, 1),
            "w2a": np.asarray(inputs["conv1_w2"], np.float32),
            "b2a": np.asarray(inputs["conv1_b2"], np.float32).reshape(C.H, 1),
            "cw1": np.concatenate(list(np.asarray(inputs["convs_w1"], np.float32)), axis=1),
            "cb1": np.asarray(inputs["convs_b1"], np.float32).T.copy(),
            "cw2": np.concatenate(list(np.asarray(inputs["convs_w2"], np.float32)), axis=1),
            "cb2": np.asarray(inputs["convs_b2"], np.float32).T.copy(),
            "l1w": np.asarray(inputs["lin1_w"], np.float32),
            "l1b": np.asarray(inputs["lin1_b"], np.float32).reshape(C.H, 1),
            "l2w": np.asarray(inputs["lin2_w"], np.float32),
            "l2b": np.asarray(inputs["lin2_b"], np.float32).reshape(C.OUT, 1),
            "iota": np.tile(np.arange(512, dtype=np.float32), (128, 1)),
            "ident": np.eye(128, dtype=np.float32),
        }
        in_maps.append(m)

    counts = {"NG": NG, "NGTOT": NGTOT, "NBTOT": NBTOT}
    return in_maps, counts


# ----------------------------------------------------------------------------
# Device module
# ----------------------------------------------------------------------------

def _build(C: Cfg, counts, repeat=1):
    ABL = set(os.environ.get("KABL", "").split(","))
    from concourse.tile import add_dep_helper

    NG, NGTOT, NBTOT = counts["NG"], counts["NGTOT"], counts["NBTOT"]
    NL = 4  # GIN layers
    DVE = mybir.EngineType.DVE
    AF = mybir.ActivationFunctionType
    AL = mybir.AluOpType
    BCOLS = C.BG * C.GT * 128 // 16     # idx cols per gather batch
    GCOLS = C.BG * C.GT                 # dstl/xsrc cols per batch
    NIDX = C.BG * C.GT * 128            # idxs per gather

    LOCAL = os.environ.get("KLOCAL") == "1"
    nc = bacc.Bacc("TRN2", target_bir_lowering=False, debug=False,
                   enable_asserts=False,
                   num_devices=1 if LOCAL else C.CORES)

    def ein(name, shape, dt=F32):
        return nc.dram_tensor(name, shape, dt, kind="ExternalInput").ap()

    IDX = ein("g_idx", [128, NBTOT * BCOLS], I16)
    DSTL = ein("g_dstl", [128, NGTOT * C.GT])
    XSRC = ein("g_xsrc", [128, NGTOT * C.GT])
    FLUSH = ein("g_flush", [1, NGTOT], I32)
    XOWN = ein("xown", [1, C.NPC_PAD])
    N2S = ein("n2s", [128, C.NT_REAL])
    S2G = ein("s2g", [128, C.S_T])
    SSTART = ein("sstart", [1, 1], I32)
    W1A, B1A = ein("w1a", [1, C.H]), ein("b1a", [C.H, 1])
    W2A, B2A = ein("w2a", [C.H, C.H]), ein("b2a", [C.H, 1])
    CW1, CB1 = ein("cw1", [C.H, 3 * C.H]), ein("cb1", [C.H, 3])
    CW2, CB2 = ein("cw2", [C.H, 3 * C.H]), ein("cb2", [C.H, 3])
    L1W, L1B = ein("l1w", [C.H, C.H]), ein("l1b", [C.H, 1])
    L2W, L2B = ein("l2w", [C.H, C.OUT]), ein("l2b", [C.OUT, 1])
    IOTA = ein("iota", [128, 512])
    IDENT = ein("ident", [128, 128])
    OUTT = nc.dram_tensor("out", [C.NGRAPH, C.OUT], F32, kind="ExternalOutput").ap()

    hbuf = [nc.dram_tensor(f"hbuf{k}", [C.N, C.H], F32, kind="Internal",
                           addr_space="Shared").ap() for k in range(2)]
    agin = [nc.dram_tensor(f"agin{k}", [C.NPC, C.H], F32, kind="Internal").ap()
            for k in range(2)]
    ppin = nc.dram_tensor("ppin", [C.H, C.SPAD], F32, kind="Internal").ap()
    ppout = nc.dram_tensor("ppout", [C.H, C.SPAD], F32, kind="Internal",
                           addr_space="Shared").ap()

    RG = [list(range(C.CORES))]
    AGGW = C.NPC_PAD + C.W

    with tile.TileContext(nc) as tc:
        from concourse import library_config
        nc.gpsimd.load_library(library_config.mlp)
        with (
            tc.tile_pool(name="const", bufs=1) as P0,
            tc.tile_pool(name="stream", bufs=3) as PS,
            tc.tile_pool(name="msgs", bufs=2) as PM,
            tc.tile_pool(name="oh", bufs=4) as PO,
            tc.tile_pool(name="mlp", bufs=3) as PL,
            tc.tile_pool(name="ps_sc", bufs=3, space="PSUM") as PSC,
            tc.tile_pool(name="ps_mlp", bufs=2, space="PSUM") as PSM,
            tc.tile_pool(name="ps_tr", bufs=2, space="PSUM") as PST,
            tc.tile_pool(name="ps_pool", bufs=1, space="PSUM") as PSP,
        ):
            # ---- resident tiles ----
            def load(ap, shape, dt=F32, tag=None):
                t = P0.tile(shape, dt, tag=tag)
                nc.sync.dma_start(t[:], ap)
                return t

            flush_sb = load(FLUSH, [1, NGTOT], I32, tag="c_flush")
            n2s_sb = load(N2S, [128, C.NT_REAL], tag="c_n2s")
            s2g_sb = load(S2G, [128, C.S_T], tag="c_s2g")
            sstart_sb = load(SSTART, [1, 1], I32, tag="c_sstart")
            w1a_sb, b1a_sb = load(W1A, [1, C.H], tag="c_w1a"), load(B1A, [C.H, 1], tag="c_b1a")
            w2a_sb, b2a_sb = load(W2A, [C.H, C.H], tag="c_w2a"), load(B2A, [C.H, 1], tag="c_b2a")
            cw1_sb, cb1_sb = load(CW1, [C.H, 3 * C.H], tag="c_cw1"), load(CB1, [C.H, 3], tag="c_cb1")
            cw2_sb, cb2_sb = load(CW2, [C.H, 3 * C.H], tag="c_cw2"), load(CB2, [C.H, 3], tag="c_cb2")
            l1w_sb, l1b_sb = load(L1W, [C.H, C.H], tag="c_l1w"), load(L1B, [C.H, 1], tag="c_l1b")
            l2w_sb, l2b_sb = load(L2W, [C.H, C.OUT], tag="c_l2w"), load(L2B, [C.OUT, 1], tag="c_l2b")
            iota_sb = load(IOTA, [128, 512], tag="c_iota")
            ident_sb = load(IDENT, [128, 128], tag="c_ident")

            agg = P0.tile([C.H, AGGW], F32, tag="agg")
            hT = P0.tile([C.H, C.NPC_PAD], F32, tag="hT")
            expb = P0.tile([128, C.NT_REAL * C.H], F32, tag="expb")
            pp_sb = P0.tile([C.H, C.SPAD], F32, tag="pp_sb")
            p_sb = P0.tile([C.H, C.SPAD], F32, tag="p_sb")

            regs = [nc.alloc_registers(f"rof{k}", engines=[DVE]) for k in range(4)]
            sreg = nc.alloc_registers("sreg", engines=[DVE])

            ag_inst = None  # last AllGather instruction (DRAM dep anchor)

            for _rep in range(repeat):
              for layer in range(NL):
                  l0 = layer == 0
                  nc.vector.memset(agg[:], 0.0)
                  if l0:
                      w1, b1, w2, b2 = w1a_sb, b1a_sb, w2a_sb, b2a_sb
                  else:
                      r0 = (layer - 1) * C.H
                      li = layer - 1
                      w1 = cw1_sb[:, r0:r0 + C.H]
                      b1 = cb1_sb[:, li:li + 1]
                      w2 = cw2_sb[:, r0:r0 + C.H]
                      b2 = cb2_sb[:, li:li + 1]
                  src_hbuf = hbuf[(layer + 1) % 2] if not l0 else None

                  # ---------- aggregation ----------
                  g_i = 0
                  b_i = 0
                  for cv in range(C.NCH):
                      view = None
                      if not l0:
                          view = src_hbuf[cv * C.CHUNK:(cv + 1) * C.CHUNK, :]
                      for b in range(NG[cv] // C.BG):
                          if l0:
                              mt = None
                              xt = PS.tile([128, GCOLS], F32, tag="xsrc_t")
                              nc.sync.dma_start(
                                  xt[:], XSRC[:, g_i * C.GT:g_i * C.GT + GCOLS])
                          else:
                              it = PS.tile([128, BCOLS], I16, tag="idx_t")
                              nc.sync.dma_start(
                                  it[:], IDX[:, b_i * BCOLS:(b_i + 1) * BCOLS])
                              mt = PM.tile([128, GCOLS, C.H], F32, tag="msgs")
                              gi = nc.gpsimd.dma_gather(
                                  mt[:], view, it[:], NIDX, NIDX, C.H,
                                  single_packet=False)
                              if ag_inst is not None:
                                  add_dep_helper(gi.ins, ag_inst.ins,
                                                 reason="gather after allgather")
                          dt_t = PS.tile([128, GCOLS], F32, tag="dstl_t")
                          nc.sync.dma_start(
                              dt_t[:], DSTL[:, g_i * C.GT:g_i * C.GT + GCOLS])
                          for gg in range(C.BG):
                              ps = PSC.tile([1 if l0 else C.H, C.W], F32, tag="ps_sc")
                              for t in range(C.GT):
                                  tcol = gg * C.GT + t
                                  oh = PO.tile([128, C.W], F32, tag="oh")
                                  nc.vector.tensor_scalar(
                                      oh[:], iota_sb[:, :C.W],
                                      dt_t[:, tcol:tcol + 1], None, AL.is_equal)
                                  lhsT = (xt[:, tcol:tcol + 1] if l0 else
                                          mt[:, tcol, :])
                                  nc.tensor.matmul(ps[:], lhsT, oh[:],
                                                   start=(t == 0), stop=(t == C.GT - 1))
                              r = regs[g_i % 4]
                              nc.vector.reg_load(r, flush_sb[0:1, g_i:g_i + 1])
                              off = nc.vector.snap(r, donate=False, min_val=0,
                                                   max_val=C.NPC)
                              dyn = agg[0:1, bass.ds(off, C.W)] if l0 else \
                                  agg[:, bass.ds(off, C.W)]
                              nc.vector.tensor_tensor(dyn, ps[:], dyn, AL.add)
                              g_i += 1
                          b_i += 1
                  assert g_i == NGTOT

                  # ---------- update (h + agg -> MLP) ----------
                  for ngp in range(C.NMG):
                      sl = slice(ngp * 512, (ngp + 1) * 512)
                      if l0:
                          xo = PL.tile([1, 512], F32, tag="xo")
                          nc.sync.dma_start(xo[:], XOWN[0:1, sl])
                          hin = PL.tile([1, 512], F32, tag="hin0")
                          nc.vector.tensor_tensor(hin[:], xo[:], agg[0:1, sl], AL.add)
                      else:
                          hin = PL.tile([C.H, 512], F32, tag="hin")
                          nc.vector.tensor_tensor(hin[:], hT[:, sl], agg[:, sl], AL.add)
                      ps1 = PSM.tile([C.H, 512], F32, tag="ps_mlp")
                      nc.tensor.matmul(ps1[:], w1, hin[:], start=True, stop=True)
                      t1 = PL.tile([C.H, 512], F32, tag="t1")
                      nc.scalar.activation(t1[:], ps1[:], AF.Relu, bias=b1)
                      ps2 = PSM.tile([C.H, 512], F32, tag="ps_mlp")
                      nc.tensor.matmul(ps2[:], w2, t1[:], start=True, stop=True)
                      nc.scalar.activation(hT[:, sl], ps2[:], AF.Relu, bias=b2)

                  # ---------- transpose to node-major ----------
                  last = layer == NL - 1
                  if last:
                      ps_s = PSP.tile([C.H, C.SWIN], F32, tag="ps_pool")
                  for j in range(C.NT_REAL):
                      pt = PST.tile([128, C.H], F32, tag="ps_tr")
                      nc.tensor.transpose(pt[:], hT[:, j * 128:(j + 1) * 128],
                                          ident_sb[:C.H, :C.H])
                      nc.scalar.activation(expb[:, j * C.H:(j + 1) * C.H], pt[:],
                                           AF.Copy)
                      if last:
                          ohs = PO.tile([128, C.SWIN], F32, tag="ohs")
                          nc.vector.tensor_scalar(ohs[:], iota_sb[:, :C.SWIN],
                                                  n2s_sb[:, j:j + 1], None,
                                                  AL.is_equal)
                          nc.tensor.matmul(ps_s[:],
                                           expb[:, j * C.H:(j + 1) * C.H],
                                           ohs[:], start=(j == 0),
                                           stop=(j == C.NT_REAL - 1))

                  if not last:
                      # export + AllGather
                      dst = agin[layer % 2]
                      nf = C.NT_REAL - 1
                      d1 = nc.sync.dma_start(
                          dst[0:nf * 128, :].rearrange("(b p) f -> p b f", p=128),
                          expb[:, :nf * C.H].rearrange("p (b f) -> p b f", f=C.H))
                      d2 = nc.sync.dma_start(
                          dst[nf * 128:C.NPC, :],
                          expb[:C.LAST_ROWS, nf * C.H:(nf + 1) * C.H])
                      if LOCAL:
                          ag = nc.sync.dma_start(
                              hbuf[layer % 2][0:C.NPC, :], dst)
                      else:
                          ag = nc.gpsimd.collective_compute(
                              "AllGather", AL.bypass, replica_groups=RG,
                              ins=[dst], outs=[hbuf[layer % 2]])
                      add_dep_helper(ag.ins, d1.ins, reason="ag after export")
                      add_dep_helper(ag.ins, d2.ins, reason="ag after export")
                      ag_inst = ag

            # ---------- subgraph partial sums -> AllReduce ----------
            nc.vector.memset(pp_sb[:], 0.0)
            nc.vector.reg_load(sreg, sstart_sb[0:1, 0:1])
            soff = nc.vector.snap(sreg, donate=True, min_val=0,
                                  max_val=C.SPAD - C.SWIN)
            dynp = pp_sb[:, bass.ds(soff, C.SWIN)]
            nc.vector.tensor_copy(dynp, ps_s[:])
            d3 = nc.sync.dma_start(ppin, pp_sb[:])
            if LOCAL:
                ar = nc.sync.dma_start(ppout, ppin)
            else:
                ar = nc.gpsimd.collective_compute(
                    "AllReduce", AL.add, replica_groups=RG, ins=[ppin], outs=[ppout])
            add_dep_helper(ar.ins, d3.ins, reason="ar after store")
            d4 = nc.sync.dma_start(p_sb[:], ppout)
            add_dep_helper(d4.ins, ar.ins, reason="load after ar")

            # ---------- graph pooling ----------
            ps_g = PSP.tile([C.H, C.NGRAPH], F32, tag="ps_pool")
            for jt in range(C.S_T):
                pt = PST.tile([128, C.H], F32, tag="ps_tr")
                nc.tensor.transpose(pt[:], p_sb[:, jt * 128:(jt + 1) * 128],
                                    ident_sb[:C.H, :C.H])
                sm = PL.tile([128, C.H], F32, tag="smaj")
                nc.scalar.activation(sm[:], pt[:], AF.Copy)
                ohg = PO.tile([128, C.NGRAPH], F32, tag="ohg")
                nc.vector.tensor_scalar(ohg[:], iota_sb[:, :C.NGRAPH],
                                        s2g_sb[:, jt:jt + 1], None, AL.is_equal)
                nc.tensor.matmul(ps_g[:], sm[:], ohg[:], start=(jt == 0),
                                 stop=(jt == C.S_T - 1))

            # ---------- head ----------
            g_sb = PL.tile([C.H, C.NGRAPH], F32, tag="gsb")
            nc.scalar.activation(g_sb[:], ps_g[:], AF.Copy)
            ph1 = PSM.tile([C.H, C.NGRAPH], F32, tag="ps_mlp")
            nc.tensor.matmul(ph1[:], l1w_sb[:], g_sb[:], start=True, stop=True)
            t1h = PL.tile([C.H, C.NGRAPH], F32, tag="t1h")
            nc.scalar.activation(t1h[:], ph1[:], AF.Relu, bias=l1b_sb[:])
            ph2 = PSM.tile([C.OUT, C.NGRAPH], F32, tag="ps_mlp")
            nc.tensor.matmul(ph2[:], l2w_sb[:], t1h[:], start=True, stop=True)
            t2h = PL.tile([C.OUT, C.NGRAPH], F32, tag="t2h")
            nc.scalar.activation(t2h[:], ph2[:], AF.Identity, bias=l2b_sb[:])

            # ---------- log_softmax over classes ----------
            ptz = PST.tile([128, C.OUT], F32, tag="ps_tr")
            nc.tensor.transpose(ptz[:C.NGRAPH, :], t2h[:],
                                ident_sb[:C.OUT, :C.OUT])
            z = PL.tile([C.NGRAPH, C.OUT], F32, tag="z")
            nc.scalar.activation(z[:], ptz[:C.NGRAPH, :], AF.Copy)
            mx = PL.tile([C.NGRAPH, 1], F32, tag="mx")
            nc.vector.tensor_reduce(mx[:], z[:], mybir.AxisListType.X, AL.max)
            zc = PL.tile([C.NGRAPH, C.OUT], F32, tag="zc")
            nc.vector.tensor_scalar(zc[:], z[:], mx[:], None, AL.subtract)
            ex = PL.tile([C.NGRAPH, C.OUT], F32, tag="ex")
            nc.scalar.activation(ex[:], zc[:], AF.Exp)
            sm2 = PL.tile([C.NGRAPH, 1], F32, tag="sm2")
            nc.vector.tensor_reduce(sm2[:], ex[:], mybir.AxisListType.X, AL.add)
            ls = PL.tile([C.NGRAPH, 1], F32, tag="ls")
            nc.scalar.activation(ls[:], sm2[:], AF.Ln)
            res = PL.tile([C.NGRAPH, C.OUT], F32, tag="res")
            nc.vector.tensor_scalar(res[:], zc[:], ls[:], None, AL.subtract)
            nc.sync.dma_start(OUTT, res[:])

    nc.compile()
    return nc


# ----------------------------------------------------------------------------
# Runner
# ----------------------------------------------------------------------------

_CACHE = {}


def _run_sim(nc, in_maps, C: Cfg):
    from concourse.bass_interp import MultiCoreSim
    sim = MultiCoreSim(nc, num_cores=C.CORES, trace=False,
                       require_finite=False, require_nnan=False)
    for c in range(C.CORES):
        for k, v in in_maps[c].items():
            sim.cores[c].tensor(k)[:] = v
    sim.simulate(check_with_hw=False)
    return np.array(sim.cores[0].mem_tensor("out"))


def _run_hw(nc, in_maps, C: Cfg, trace=False, tmpdir=None):
    from concourse.bass_utils import run_bass_kernel_spmd
    res = run_bass_kernel_spmd(nc, in_maps, core_ids=list(range(C.CORES)),
                               trace=trace, tmpdir=tmpdir)
    return res.results[0]["out"], res


def kernel(**inputs):
    C = Cfg()
    in_maps, counts = _prep(inputs, C)
    key = ("full", tuple(counts["NG"]))
    if key not in _CACHE:
        _CACHE[key] = _build(C, counts)
    out, _ = _run_hw(_CACHE[key], in_maps, C)
    return np.asarray(out, np.float32)



# revision 6
# speedup vs baseline: 1.0170x; 1.0170x over previous
"""NestedGIN (4-layer GIN + 2-level pooling + MLP head) on 8 Trainium2 NeuronCores.

Strategy:
  - Nodes (and their incident in-edges, i.e. edges grouped by dst) are sharded
    across 8 cores; MLP weights are replicated.
  - Per layer: each core gathers h[src] for its edges from a replicated
    node-major copy of h in HBM (hardware dma_gather, 256B rows), then
    scatter-adds into its node slice via one-hot matmuls on the TensorEngine
    (edges sorted by dst; 128-edge tiles vs 128-node windows; psum窗 flushed
    into a feature-major SBUF accumulator at data-driven dynamic offsets).
  - The GIN MLP runs feature-major ([64, nodes] tiles, weights stationary).
  - The updated slice is transposed back to node-major, exported to HBM and
    AllGather'ed for the next layer's gathers.
  - Final: subgraph pooling (one-hot matmul vs node_to_subgraph) -> AllReduce
    of partial subgraph sums -> graph pooling -> MLP head -> log_softmax.

Host-side numpy does only index/layout prep (sharding, sorting, padding) plus
the layer-0 input-feature reindex x[src] (pure copy, no arithmetic).
"""

import os
import sys

for _p in ("/opt/trn_rl_repo", "/opt/pypackages"):
    if os.path.isdir(_p) and _p not in sys.path:
        sys.path.append(_p)

import numpy as np

import concourse.bass as bass
import concourse.bacc as bacc
import concourse.tile as tile
import concourse.mybir as mybir

F32 = mybir.dt.float32
I32 = mybir.dt.int32
I16 = mybir.dt.int16


class Cfg:
    def __init__(self, N=100000, E=1600000, S=2000, NGRAPH=64, OUT=8,
                 CORES=8, CHUNK=25000):
        self.N, self.E, self.S = N, S and N and E, S  # keep E explicit below
        self.E = E
        self.NGRAPH, self.OUT, self.CORES = NGRAPH, OUT, CORES
        self.H = 64
        self.NPC = N // CORES                      # nodes per core
        assert N % CORES == 0
        self.CHUNK = CHUNK                         # gather-source chunk rows (int16 idx limit)
        assert CHUNK <= 32767
        self.NCH = (N + CHUNK - 1) // CHUNK        # chunks
        assert N % CHUNK == 0
        self.GT = 4                                # edge tiles (128 edges) per group
        self.BG = 8                                # groups per dma_gather batch
        self.W = 160                               # node window per group
        self.NPC_PAD = ((self.NPC + 511) // 512) * 512
        self.NMG = self.NPC_PAD // 512             # MLP node-groups of 512
        self.NT_REAL = (self.NPC + 127) // 128     # node-major tiles (transpose/export)
        self.LAST_ROWS = self.NPC - (self.NT_REAL - 1) * 128
        self.SPAD = 2048                           # padded #subgraphs
        assert S <= self.SPAD
        self.SWIN = 512                            # per-core subgraph window
        self.S_T = self.SPAD // 128
        assert NGRAPH <= 64


# ----------------------------------------------------------------------------
# Host-side prep: shard + sort edges, build tiles/groups, pack device inputs.
# ----------------------------------------------------------------------------

def _prep(inputs, C: Cfg):
    x = np.asarray(inputs["x"], np.float32).reshape(C.N)
    ei = np.asarray(inputs["edge_index"]).astype(np.int64)
    n2s = np.asarray(inputs["node_to_subgraph"]).astype(np.int64)
    s2g = np.asarray(inputs["subgraph_to_graph"]).astype(np.int64)
    src_all, dst_all = ei[0], ei[1]

    # -------- per-core edge groups --------
    per_core = []  # per core: list over chunks of list of groups
    for c in range(C.CORES):
        lo = c * C.NPC
        m = (dst_all >= lo) & (dst_all < lo + C.NPC)
        s, d = src_all[m], dst_all[m]
        ch = s // C.CHUNK
        o = np.lexsort((s, d, ch))
        s, d, ch = s[o], d[o], ch[o]
        chunks = []
        for cv in range(C.NCH):
            sel = ch == cv
            ss, dd = s[sel], d[sel]
            groups = []
            i, n = 0, len(ss)
            cap = C.GT * 128
            while i < n:
                j = min(i + cap, n)
                # keep node span < W
                if dd[j - 1] - dd[i] >= C.W:
                    j = int(np.searchsorted(dd, dd[i] + C.W, side="left"))
                base = int(dd[i])
                gs = ss[i:j]
                gd = dd[i:j] - base
                pad = cap - (j - i)
                if pad:
                    gs = np.concatenate([gs, np.zeros(pad, np.int64) + cv * C.CHUNK])
                    gd = np.concatenate([gd, np.full(pad, -1, np.int64)])
                groups.append((base - lo, gs, gd))
                i = j
            chunks.append(groups)
        per_core.append(chunks)

    # uniform group counts per chunk across cores (SPMD)
    NG = []
    for cv in range(C.NCH):
        mx = max(len(per_core[c][cv]) for c in range(C.CORES))
        mx = ((mx + C.BG - 1) // C.BG) * C.BG
        NG.append(mx)
    NGTOT = sum(NG)
    NBTOT = NGTOT // C.BG
    cap = C.GT * 128

    pad_group = (C.NPC, np.zeros(cap, np.int64), np.full(cap, -1, np.int64))

    in_maps = []
    for c in range(C.CORES):
        lo = c * C.NPC
        dstl = np.empty((128, NGTOT * C.GT), np.float32)
        xsrc = np.zeros((128, NGTOT * C.GT), np.float32)
        flush = np.empty((1, NGTOT), np.int32)
        bcols = C.BG * cap // 16
        idx = np.empty((16, NBTOT * bcols), np.int16)  # replicated to 128 below
        g_i = 0
        for cv in range(C.NCH):
            groups = per_core[c][cv]
            for k in range(NG[cv]):
                off, gs, gd = groups[k] if k < len(groups) else \
                    (pad_group[0], pad_group[1] + cv * C.CHUNK, pad_group[2])
                flush[0, g_i] = off
                dstl[:, g_i * C.GT:(g_i + 1) * C.GT] = \
                    gd.reshape(C.GT, 128).T.astype(np.float32)
                xv = x[gs]
                xv[gd < 0] = 0.0
                xsrc[:, g_i * C.GT:(g_i + 1) * C.GT] = xv.reshape(C.GT, 128).T
                # wrapped int16 idx layout for this group's slot of its batch
                loc = (gs - cv * C.CHUNK).astype(np.int16)
                b = g_i // C.BG
                half = g_i % C.BG
                gcols = cap // 16
                blk = loc.reshape(gcols, 16).T
                idx[:, b * bcols + half * gcols:b * bcols + (half + 1) * gcols] = blk
                g_i += 1
        assert g_i == NGTOT

        own_n2s = n2s[lo:lo + C.NPC]
        sstart = int(min(max(0, own_n2s.min()), C.SPAD - C.SWIN))
        assert own_n2s.max() - sstart < C.SWIN, "subgraph window overflow"
        n2s_lin = np.full(C.NT_REAL * 128, -1, np.float32)
        n2s_lin[:C.NPC] = own_n2s - sstart
        n2s_loc = n2s_lin.reshape(C.NT_REAL, 128).T

        s2g_lin = np.full(C.SPAD, -1, np.float32)
        s2g_lin[:C.S] = s2g
        s2g_loc = s2g_lin.reshape(C.S_T, 128).T

        xown = np.zeros((1, C.NPC_PAD), np.float32)
        xown[0, :C.NPC] = x[lo:lo + C.NPC]

        m = {
            "g_idx": np.tile(idx, (8, 1)), "g_dstl": dstl, "g_xsrc": xsrc, "g_flush": flush,
            "xown": xown, "n2s": n2s_loc.astype(np.float32),
            "s2g": s2g_loc.astype(np.float32),
            "sstart": np.array([[sstart]], np.int32),
            "w1a": np.asarray(inputs["conv1_w1"], np.float32),
            "b1a": np.asarray(inputs["conv1_b1"], np.float32).reshape(C.H, 1),
            "w2a": np.asarray(inputs["conv1_w2"], np.float32),
            "b2a": np.asarray(inputs["conv1_b2"], np.float32).reshape(C.H, 1),
            "cw1": np.concatenate(list(np.asarray(inputs["convs_w1"], np.float32)), axis=1),
            "cb1": np.asarray(inputs["convs_b1"], np.float32).T.copy(),
            "cw2": np.concatenate(list(np.asarray(inputs["convs_w2"], np.float32)), axis=1),
            "cb2": np.asarray(inputs["convs_b2"], np.float32).T.copy(),
            "l1w": np.asarray(inputs["lin1_w"], np.float32),
            "l1b": np.asarray(inputs["lin1_b"], np.float32).reshape(C.H, 1),
            "l2w": np.asarray(inputs["lin2_w"], np.float32),
            "l2b": np.asarray(inputs["lin2_b"], np.float32).reshape(C.OUT, 1),
            "iota": np.tile(np.arange(512, dtype=np.float32), (128, 1)),
            "ident": np.eye(128, dtype=np.float32),
        }
        in_maps.append(m)

    counts = {"NG": NG, "NGTOT": NGTOT, "NBTOT": NBTOT}
    return in_maps, counts


# ----------------------------------------------------------------------------
# Device module
# ----------------------------------------------------------------------------

def _build(C: Cfg, counts, repeat=1):
    ABL = set(os.environ.get("KABL", "").split(","))
    from concourse.tile import add_dep_helper

    NG, NGTOT, NBTOT = counts["NG"], counts["NGTOT"], counts["NBTOT"]
    NL = 4  # GIN layers
    DVE = mybir.EngineType.DVE
    AF = mybir.ActivationFunctionType
    AL = mybir.AluOpType
    BCOLS = C.BG * C.GT * 128 // 16     # idx cols per gather batch
    GCOLS = C.BG * C.GT                 # dstl/xsrc cols per batch
    NIDX = C.BG * C.GT * 128            # idxs per gather

    LOCAL = os.environ.get("KLOCAL") == "1"
    nc = bacc.Bacc("TRN2", target_bir_lowering=False, debug=False,
                   enable_asserts=False, num_swdge_queues=4,
                   num_devices=1 if LOCAL else C.CORES)

    def ein(name, shape, dt=F32):
        return nc.dram_tensor(name, shape, dt, kind="ExternalInput").ap()

    IDX = ein("g_idx", [128, NBTOT * BCOLS], I16)
    DSTL = ein("g_dstl", [128, NGTOT * C.GT])
    XSRC = ein("g_xsrc", [128, NGTOT * C.GT])
    FLUSH = ein("g_flush", [1, NGTOT], I32)
    XOWN = ein("xown", [1, C.NPC_PAD])
    N2S = ein("n2s", [128, C.NT_REAL])
    S2G = ein("s2g", [128, C.S_T])
    SSTART = ein("sstart", [1, 1], I32)
    W1A, B1A = ein("w1a", [1, C.H]), ein("b1a", [C.H, 1])
    W2A, B2A = ein("w2a", [C.H, C.H]), ein("b2a", [C.H, 1])
    CW1, CB1 = ein("cw1", [C.H, 3 * C.H]), ein("cb1", [C.H, 3])
    CW2, CB2 = ein("cw2", [C.H, 3 * C.H]), ein("cb2", [C.H, 3])
    L1W, L1B = ein("l1w", [C.H, C.H]), ein("l1b", [C.H, 1])
    L2W, L2B = ein("l2w", [C.H, C.OUT]), ein("l2b", [C.OUT, 1])
    IOTA = ein("iota", [128, 512])
    IDENT = ein("ident", [128, 128])
    OUTT = nc.dram_tensor("out", [C.NGRAPH, C.OUT], F32, kind="ExternalOutput").ap()

    hbuf = [nc.dram_tensor(f"hbuf{k}", [C.N, C.H], F32, kind="Internal",
                           addr_space="Shared").ap() for k in range(2)]
    agin = [nc.dram_tensor(f"agin{k}", [C.NPC, C.H], F32, kind="Internal").ap()
            for k in range(2)]
    ppin = nc.dram_tensor("ppin", [C.H, C.SPAD], F32, kind="Internal").ap()
    ppout = nc.dram_tensor("ppout", [C.H, C.SPAD], F32, kind="Internal",
                           addr_space="Shared").ap()

    RG = [list(range(C.CORES))]
    AGGW = C.NPC_PAD + C.W

    with tile.TileContext(nc) as tc:
        from concourse import library_config
        nc.gpsimd.load_library(library_config.mlp)
        with (
            tc.tile_pool(name="const", bufs=1) as P0,
            tc.tile_pool(name="stream", bufs=4) as PS,
            tc.tile_pool(name="msgs", bufs=2) as PM,
            tc.tile_pool(name="oh", bufs=4) as PO,
            tc.tile_pool(name="mlp", bufs=3) as PL,
            tc.tile_pool(name="ps_sc", bufs=3, space="PSUM") as PSC,
            tc.tile_pool(name="ps_mlp", bufs=2, space="PSUM") as PSM,
            tc.tile_pool(name="ps_tr", bufs=2, space="PSUM") as PST,
            tc.tile_pool(name="ps_pool", bufs=1, space="PSUM") as PSP,
        ):
            # ---- resident tiles ----
            def load(ap, shape, dt=F32, tag=None):
                t = P0.tile(shape, dt, tag=tag)
                nc.sync.dma_start(t[:], ap)
                return t

            flush_sb = load(FLUSH, [1, NGTOT], I32, tag="c_flush")
            n2s_sb = load(N2S, [128, C.NT_REAL], tag="c_n2s")
            s2g_sb = load(S2G, [128, C.S_T], tag="c_s2g")
            sstart_sb = load(SSTART, [1, 1], I32, tag="c_sstart")
            w1a_sb, b1a_sb = load(W1A, [1, C.H], tag="c_w1a"), load(B1A, [C.H, 1], tag="c_b1a")
            w2a_sb, b2a_sb = load(W2A, [C.H, C.H], tag="c_w2a"), load(B2A, [C.H, 1], tag="c_b2a")
            cw1_sb, cb1_sb = load(CW1, [C.H, 3 * C.H], tag="c_cw1"), load(CB1, [C.H, 3], tag="c_cb1")
            cw2_sb, cb2_sb = load(CW2, [C.H, 3 * C.H], tag="c_cw2"), load(CB2, [C.H, 3], tag="c_cb2")
            l1w_sb, l1b_sb = load(L1W, [C.H, C.H], tag="c_l1w"), load(L1B, [C.H, 1], tag="c_l1b")
            l2w_sb, l2b_sb = load(L2W, [C.H, C.OUT], tag="c_l2w"), load(L2B, [C.OUT, 1], tag="c_l2b")
            iota_sb = load(IOTA, [128, 512], tag="c_iota")
            ident_sb = load(IDENT, [128, 128], tag="c_ident")

            agg = P0.tile([C.H, AGGW], F32, tag="agg")
            hT = P0.tile([C.H, C.NPC_PAD], F32, tag="hT")
            expb = P0.tile([128, C.NT_REAL * C.H], F32, tag="expb")
            pp_sb = P0.tile([C.H, C.SPAD], F32, tag="pp_sb")
            p_sb = P0.tile([C.H, C.SPAD], F32, tag="p_sb")

            regs = [nc.alloc_registers(f"rof{k}", engines=[DVE]) for k in range(4)]
            sreg = nc.alloc_registers("sreg", engines=[DVE])

            ag_inst = None  # last AllGather instruction (DRAM dep anchor)

            for _rep in range(repeat):
              for layer in range(NL):
                  l0 = layer == 0
                  nc.vector.memset(agg[:], 0.0)
                  if l0:
                      w1, b1, w2, b2 = w1a_sb, b1a_sb, w2a_sb, b2a_sb
                  else:
                      r0 = (layer - 1) * C.H
                      li = layer - 1
                      w1 = cw1_sb[:, r0:r0 + C.H]
                      b1 = cb1_sb[:, li:li + 1]
                      w2 = cw2_sb[:, r0:r0 + C.H]
                      b2 = cb2_sb[:, li:li + 1]
                  src_hbuf = hbuf[(layer + 1) % 2] if not l0 else None

                  # ---------- aggregation ----------
                  g_i = 0
                  b_i = 0
                  for cv in range(C.NCH):
                      view = None
                      if not l0:
                          view = src_hbuf[cv * C.CHUNK:(cv + 1) * C.CHUNK, :]
                      for b in range(NG[cv] // C.BG):
                          if l0:
                              mt = None
                              xt = PS.tile([128, GCOLS], F32, tag="xsrc_t")
                              nc.sync.dma_start(
                                  xt[:], XSRC[:, g_i * C.GT:g_i * C.GT + GCOLS])
                          else:
                              it = PS.tile([128, BCOLS], I16, tag="idx_t")
                              nc.sync.dma_start(
                                  it[:], IDX[:, b_i * BCOLS:(b_i + 1) * BCOLS])
                              mt = PM.tile([128, GCOLS, C.H], F32, tag="msgs")
                              gi = nc.gpsimd.dma_gather(
                                  mt[:], view, it[:], NIDX, NIDX, C.H,
                                  single_packet=False, queue_num=b_i % 4)
                              if ag_inst is not None:
                                  add_dep_helper(gi.ins, ag_inst.ins,
                                                 reason="gather after allgather")
                          dt_t = PS.tile([128, GCOLS], F32, tag="dstl_t")
                          nc.sync.dma_start(
                              dt_t[:], DSTL[:, g_i * C.GT:g_i * C.GT + GCOLS])
                          for gg in range(C.BG):
                              ps = PSC.tile([1 if l0 else C.H, C.W], F32, tag="ps_sc")
                              for t in range(C.GT):
                                  tcol = gg * C.GT + t
                                  oh = PO.tile([128, C.W], F32, tag="oh")
                                  nc.vector.tensor_scalar(
                                      oh[:], iota_sb[:, :C.W],
                                      dt_t[:, tcol:tcol + 1], None, AL.is_equal)
                                  lhsT = (xt[:, tcol:tcol + 1] if l0 else
                                          mt[:, tcol, :])
                                  nc.tensor.matmul(ps[:], lhsT, oh[:],
                                                   start=(t == 0), stop=(t == C.GT - 1))
                              r = regs[g_i % 4]
                              nc.vector.reg_load(r, flush_sb[0:1, g_i:g_i + 1])
                              off = nc.vector.snap(r, donate=False, min_val=0,
                                                   max_val=C.NPC)
                              dyn = agg[0:1, bass.ds(off, C.W)] if l0 else \
                                  agg[:, bass.ds(off, C.W)]
                              nc.vector.tensor_tensor(dyn, ps[:], dyn, AL.add)
                              g_i += 1
                          b_i += 1
                  assert g_i == NGTOT

                  # ---------- update (h + agg -> MLP) ----------
                  for ngp in range(C.NMG):
                      sl = slice(ngp * 512, (ngp + 1) * 512)
                      if l0:
                          xo = PL.tile([1, 512], F32, tag="xo")
                          nc.sync.dma_start(xo[:], XOWN[0:1, sl])
                          hin = PL.tile([1, 512], F32, tag="hin0")
                          nc.vector.tensor_tensor(hin[:], xo[:], agg[0:1, sl], AL.add)
                      else:
                          hin = PL.tile([C.H, 512], F32, tag="hin")
                          nc.vector.tensor_tensor(hin[:], hT[:, sl], agg[:, sl], AL.add)
                      ps1 = PSM.tile([C.H, 512], F32, tag="ps_mlp")
                      nc.tensor.matmul(ps1[:], w1, hin[:], start=True, stop=True)
                      t1 = PL.tile([C.H, 512], F32, tag="t1")
                      nc.scalar.activation(t1[:], ps1[:], AF.Relu, bias=b1)
                      ps2 = PSM.tile([C.H, 512], F32, tag="ps_mlp")
                      nc.tensor.matmul(ps2[:], w2, t1[:], start=True, stop=True)
                      nc.scalar.activation(hT[:, sl], ps2[:], AF.Relu, bias=b2)

                  # ---------- transpose to node-major ----------
                  last = layer == NL - 1
                  if last:
                      ps_s = PSP.tile([C.H, C.SWIN], F32, tag="ps_pool")
                  for j in range(C.NT_REAL):
                      pt = PST.tile([128, C.H], F32, tag="ps_tr")
                      nc.tensor.transpose(pt[:], hT[:, j * 128:(j + 1) * 128],
                                          ident_sb[:C.H, :C.H])
                      nc.scalar.activation(expb[:, j * C.H:(j + 1) * C.H], pt[:],
                                           AF.Copy)
                      if last:
                          ohs = PO.tile([128, C.SWIN], F32, tag="ohs")
                          nc.vector.tensor_scalar(ohs[:], iota_sb[:, :C.SWIN],
                                                  n2s_sb[:, j:j + 1], None,
                                                  AL.is_equal)
                          nc.tensor.matmul(ps_s[:],
                                           expb[:, j * C.H:(j + 1) * C.H],
                                           ohs[:], start=(j == 0),
                                           stop=(j == C.NT_REAL - 1))

                  if not last:
                      # export + AllGather
                      dst = agin[layer % 2]
                      nf = C.NT_REAL - 1
                      d1 = nc.sync.dma_start(
                          dst[0:nf * 128, :].rearrange("(b p) f -> p b f", p=128),
                          expb[:, :nf * C.H].rearrange("p (b f) -> p b f", f=C.H))
                      d2 = nc.sync.dma_start(
                          dst[nf * 128:C.NPC, :],
                          expb[:C.LAST_ROWS, nf * C.H:(nf + 1) * C.H])
                      if LOCAL:
                          ag = nc.sync.dma_start(
                              hbuf[layer % 2][0:C.NPC, :], dst)
                      else:
                          ag = nc.gpsimd.collective_compute(
                              "AllGather", AL.bypass, replica_groups=RG,
                              ins=[dst], outs=[hbuf[layer % 2]])
                      add_dep_helper(ag.ins, d1.ins, reason="ag after export")
                      add_dep_helper(ag.ins, d2.ins, reason="ag after export")
                      ag_inst = ag

            # ---------- subgraph partial sums -> AllReduce ----------
            nc.vector.memset(pp_sb[:], 0.0)
            nc.vector.reg_load(sreg, sstart_sb[0:1, 0:1])
            soff = nc.vector.snap(sreg, donate=True, min_val=0,
                                  max_val=C.SPAD - C.SWIN)
            dynp = pp_sb[:, bass.ds(soff, C.SWIN)]
            nc.vector.tensor_copy(dynp, ps_s[:])
            d3 = nc.sync.dma_start(ppin, pp_sb[:])
            if LOCAL:
                ar = nc.sync.dma_start(ppout, ppin)
            else:
                ar = nc.gpsimd.collective_compute(
                    "AllReduce", AL.add, replica_groups=RG, ins=[ppin], outs=[ppout])
            add_dep_helper(ar.ins, d3.ins, reason="ar after store")
            d4 = nc.sync.dma_start(p_sb[:], ppout)
            add_dep_helper(d4.ins, ar.ins, reason="load after ar")

            # ---------- graph pooling ----------
            ps_g = PSP.tile([C.H, C.NGRAPH], F32, tag="ps_pool")
            for jt in range(C.S_T):
                pt = PST.tile([128, C.H], F32, tag="ps_tr")
                nc.tensor.transpose(pt[:], p_sb[:, jt * 128:(jt + 1) * 128],
                                    ident_sb[:C.H, :C.H])
                sm = PL.tile([128, C.H], F32, tag="smaj")
                nc.scalar.activation(sm[:], pt[:], AF.Copy)
                ohg = PO.tile([128, C.NGRAPH], F32, tag="ohg")
                nc.vector.tensor_scalar(ohg[:], iota_sb[:, :C.NGRAPH],
                                        s2g_sb[:, jt:jt + 1], None, AL.is_equal)
                nc.tensor.matmul(ps_g[:], sm[:], ohg[:], start=(jt == 0),
                                 stop=(jt == C.S_T - 1))

            # ---------- head ----------
            g_sb = PL.tile([C.H, C.NGRAPH], F32, tag="gsb")
            nc.scalar.activation(g_sb[:], ps_g[:], AF.Copy)
            ph1 = PSM.tile([C.H, C.NGRAPH], F32, tag="ps_mlp")
            nc.tensor.matmul(ph1[:], l1w_sb[:], g_sb[:], start=True, stop=True)
            t1h = PL.tile([C.H, C.NGRAPH], F32, tag="t1h")
            nc.scalar.activation(t1h[:], ph1[:], AF.Relu, bias=l1b_sb[:])
            ph2 = PSM.tile([C.OUT, C.NGRAPH], F32, tag="ps_mlp")
            nc.tensor.matmul(ph2[:], l2w_sb[:], t1h[:], start=True, stop=True)
            t2h = PL.tile([C.OUT, C.NGRAPH], F32, tag="t2h")
            nc.scalar.activation(t2h[:], ph2[:], AF.Identity, bias=l2b_sb[:])

            # ---------- log_softmax over classes ----------
            ptz = PST.tile([128, C.OUT], F32, tag="ps_tr")
            nc.tensor.transpose(ptz[:C.NGRAPH, :], t2h[:],
                                ident_sb[:C.OUT, :C.OUT])
            z = PL.tile([C.NGRAPH, C.OUT], F32, tag="z")
            nc.scalar.activation(z[:], ptz[:C.NGRAPH, :], AF.Copy)
            mx = PL.tile([C.NGRAPH, 1], F32, tag="mx")
            nc.vector.tensor_reduce(mx[:], z[:], mybir.AxisListType.X, AL.max)
            zc = PL.tile([C.NGRAPH, C.OUT], F32, tag="zc")
            nc.vector.tensor_scalar(zc[:], z[:], mx[:], None, AL.subtract)
            ex = PL.tile([C.NGRAPH, C.OUT], F32, tag="ex")
            nc.scalar.activation(ex[:], zc[:], AF.Exp)
            sm2 = PL.tile([C.NGRAPH, 1], F32, tag="sm2")
            nc.vector.tensor_reduce(sm2[:], ex[:], mybir.AxisListType.X, AL.add)
            ls = PL.tile([C.NGRAPH, 1], F32, tag="ls")
            nc.scalar.activation(ls[:], sm2[:], AF.Ln)
            res = PL.tile([C.NGRAPH, C.OUT], F32, tag="res")
            nc.vector.tensor_scalar(res[:], zc[:], ls[:], None, AL.subtract)
            nc.sync.dma_start(OUTT, res[:])

    nc.compile()
    return nc


# ----------------------------------------------------------------------------
# Runner
# ----------------------------------------------------------------------------

_CACHE = {}


def _run_sim(nc, in_maps, C: Cfg):
    from concourse.bass_interp import MultiCoreSim
    sim = MultiCoreSim(nc, num_cores=C.CORES, trace=False,
                       require_finite=False, require_nnan=False)
    for c in range(C.CORES):
        for k, v in in_maps[c].items():
            sim.cores[c].tensor(k)[:] = v
    sim.simulate(check_with_hw=False)
    return np.array(sim.cores[0].mem_tensor("out"))


def _run_hw(nc, in_maps, C: Cfg, trace=False, tmpdir=None):
    from concourse.bass_utils import run_bass_kernel_spmd
    res = run_bass_kernel_spmd(nc, in_maps, core_ids=list(range(C.CORES)),
                               trace=trace, tmpdir=tmpdir)
    return res.results[0]["out"], res


def kernel(**inputs):
    C = Cfg()
    in_maps, counts = _prep(inputs, C)
    key = ("full", tuple(counts["NG"]))
    if key not in _CACHE:
        _CACHE[key] = _build(C, counts)
    out, _ = _run_hw(_CACHE[key], in_maps, C)
    return np.asarray(out, np.float32)



# revision 17
# speedup vs baseline: 1.1382x; 1.1192x over previous
"""NestedGIN (4-layer GIN + 2-level pooling + MLP head) on 8 Trainium2 NeuronCores.

Strategy:
  - Nodes (and their incident in-edges, i.e. edges grouped by dst) are sharded
    across 8 cores; MLP weights are replicated.
  - Per layer: each core gathers h[src] for its edges from a replicated
    node-major copy of h in HBM (hardware dma_gather, 256B rows), then
    scatter-adds into its node slice via one-hot matmuls on the TensorEngine
    (edges sorted by dst; 128-edge tiles vs 128-node windows; psum窗 flushed
    into a feature-major SBUF accumulator at data-driven dynamic offsets).
  - The GIN MLP runs feature-major ([64, nodes] tiles, weights stationary).
  - The updated slice is transposed back to node-major, exported to HBM and
    AllGather'ed for the next layer's gathers.
  - Final: subgraph pooling (one-hot matmul vs node_to_subgraph) -> AllReduce
    of partial subgraph sums -> graph pooling -> MLP head -> log_softmax.

Host-side numpy does only index/layout prep (sharding, sorting, padding) plus
the layer-0 input-feature reindex x[src] (pure copy, no arithmetic).
"""

import os
import sys

for _p in ("/opt/trn_rl_repo", "/opt/pypackages"):
    if os.path.isdir(_p) and _p not in sys.path:
        sys.path.append(_p)

import numpy as np

import concourse.bass as bass
import concourse.bacc as bacc
import concourse.tile as tile
import concourse.mybir as mybir

F32 = mybir.dt.float32
I32 = mybir.dt.int32
I16 = mybir.dt.int16
BF16 = mybir.dt.bfloat16
F16 = mybir.dt.float16


class Cfg:
    def __init__(self, N=100000, E=1600000, S=2000, NGRAPH=64, OUT=8,
                 CORES=8, CHUNK=25000):
        self.N, self.E, self.S = N, S and N and E, S  # keep E explicit below
        self.E = E
        self.NGRAPH, self.OUT, self.CORES = NGRAPH, OUT, CORES
        self.H = 64
        self.NPC = N // CORES                      # nodes per core
        assert N % CORES == 0
        self.CHUNK = CHUNK                         # gather-source chunk rows (int16 idx limit)
        assert CHUNK <= 32767
        self.NCH = (N + CHUNK - 1) // CHUNK        # chunks
        assert N % CHUNK == 0
        self.GT = 4                                # edge tiles (128 edges) per group
        self.BG = 8                                # groups per dma_gather batch
        self.W = 160                               # node window per group
        self.NPC_PAD = ((self.NPC + 511) // 512) * 512
        self.NMG = self.NPC_PAD // 512             # MLP node-groups of 512
        self.NT_REAL = (self.NPC + 127) // 128     # node-major tiles (transpose/export)
        self.LAST_ROWS = self.NPC - (self.NT_REAL - 1) * 128
        self.SPAD = 2048                           # padded #subgraphs
        assert S <= self.SPAD
        self.SWIN = 512                            # per-core subgraph window
        self.S_T = self.SPAD // 128
        assert NGRAPH <= 64


# ----------------------------------------------------------------------------
# Host-side prep: shard + sort edges, build tiles/groups, pack device inputs.
# ----------------------------------------------------------------------------

def _prep(inputs, C: Cfg):
    x = np.asarray(inputs["x"], np.float32).reshape(C.N)
    ei = np.asarray(inputs["edge_index"]).astype(np.int64)
    n2s = np.asarray(inputs["node_to_subgraph"]).astype(np.int64)
    s2g = np.asarray(inputs["subgraph_to_graph"]).astype(np.int64)
    src_all, dst_all = ei[0], ei[1]

    # -------- per-core edge groups --------
    per_core = []  # per core: list over chunks of list of groups
    for c in range(C.CORES):
        lo = c * C.NPC
        m = (dst_all >= lo) & (dst_all < lo + C.NPC)
        s, d = src_all[m], dst_all[m]
        ch = s // C.CHUNK
        o = np.lexsort((s, d, ch))
        s, d, ch = s[o], d[o], ch[o]
        chunks = []
        for cv in range(C.NCH):
            sel = ch == cv
            ss, dd = s[sel], d[sel]
            groups = []
            i, n = 0, len(ss)
            cap = C.GT * 128
            while i < n:
                j = min(i + cap, n)
                # keep node span < W
                if dd[j - 1] - dd[i] >= C.W:
                    j = int(np.searchsorted(dd, dd[i] + C.W, side="left"))
                base = int(dd[i])
                gs = ss[i:j]
                gd = dd[i:j] - base
                pad = cap - (j - i)
                if pad:
                    gs = np.concatenate([gs, np.zeros(pad, np.int64) + cv * C.CHUNK])
                    gd = np.concatenate([gd, np.full(pad, -1, np.int64)])
                groups.append((base - lo, gs, gd))
                i = j
            chunks.append(groups)
        per_core.append(chunks)

    # uniform group counts per chunk across cores (SPMD)
    NG = []
    for cv in range(C.NCH):
        mx = max(len(per_core[c][cv]) for c in range(C.CORES))
        mx = ((mx + C.BG - 1) // C.BG) * C.BG
        NG.append(mx)
    NGTOT = sum(NG)
    NBTOT = NGTOT // C.BG
    cap = C.GT * 128

    pad_group = (C.NPC, np.zeros(cap, np.int64), np.full(cap, -1, np.int64))

    in_maps = []
    for c in range(C.CORES):
        lo = c * C.NPC
        dstl = np.empty((128, NGTOT * C.GT), np.float32)
        xsrc = np.zeros((128, NGTOT * C.GT), np.float32)
        flush = np.empty((1, NGTOT), np.int32)
        bcols = C.BG * cap // 16
        idx = np.empty((16, NBTOT * bcols), np.int16)  # replicated to 128 below
        g_i = 0
        for cv in range(C.NCH):
            groups = per_core[c][cv]
            for k in range(NG[cv]):
                off, gs, gd = groups[k] if k < len(groups) else \
                    (pad_group[0], pad_group[1] + cv * C.CHUNK, pad_group[2])
                flush[0, g_i] = off
                dstl[:, g_i * C.GT:(g_i + 1) * C.GT] = \
                    gd.reshape(C.GT, 128).T.astype(np.float32)
                xv = x[gs]
                xv[gd < 0] = 0.0
                xsrc[:, g_i * C.GT:(g_i + 1) * C.GT] = xv.reshape(C.GT, 128).T
                # wrapped int16 idx layout for this group's slot of its batch
                loc = (gs - cv * C.CHUNK).astype(np.int16)
                b = g_i // C.BG
                half = g_i % C.BG
                gcols = cap // 16
                blk = loc.reshape(gcols, 16).T
                idx[:, b * bcols + half * gcols:b * bcols + (half + 1) * gcols] = blk
                g_i += 1
        assert g_i == NGTOT

        own_n2s = n2s[lo:lo + C.NPC]
        sstart = int(min(max(0, own_n2s.min()), C.SPAD - C.SWIN))
        assert own_n2s.max() - sstart < C.SWIN, "subgraph window overflow"
        n2s_lin = np.full(C.NT_REAL * 128, -1, np.float32)
        n2s_lin[:C.NPC] = own_n2s - sstart
        n2s_loc = n2s_lin.reshape(C.NT_REAL, 128).T

        s2g_lin = np.full(C.SPAD, -1, np.float32)
        s2g_lin[:C.S] = s2g
        s2g_loc = s2g_lin.reshape(C.S_T, 128).T

        xown = np.zeros((1, C.NPC_PAD), np.float32)
        xown[0, :C.NPC] = x[lo:lo + C.NPC]

        m = {
            "g_idx": np.tile(idx, (8, 1)), "g_dstl": dstl, "g_xsrc": xsrc,
            "g_flush": flush,
            "xown": xown, "n2s": n2s_loc.astype(np.float32),
            "s2g": s2g_loc.astype(np.float32),
            "sstart": np.array([[sstart]], np.int32),
            "w1a": np.asarray(inputs["conv1_w1"], np.float32),
            "b1a": np.asarray(inputs["conv1_b1"], np.float32).reshape(C.H, 1),
            "w2a": np.asarray(inputs["conv1_w2"], np.float32),
            "b2a": np.asarray(inputs["conv1_b2"], np.float32).reshape(C.H, 1),
            "cw1": np.concatenate(list(np.asarray(inputs["convs_w1"], np.float32)), axis=1),
            "cb1": np.asarray(inputs["convs_b1"], np.float32).T.copy(),
            "cw2": np.concatenate(list(np.asarray(inputs["convs_w2"], np.float32)), axis=1),
            "cb2": np.asarray(inputs["convs_b2"], np.float32).T.copy(),
            "l1w": np.asarray(inputs["lin1_w"], np.float32),
            "l1b": np.asarray(inputs["lin1_b"], np.float32).reshape(C.H, 1),
            "l2w": np.asarray(inputs["lin2_w"], np.float32),
            "l2b": np.asarray(inputs["lin2_b"], np.float32).reshape(C.OUT, 1),
            "iota": np.tile(np.arange(512, dtype=np.float32), (128, 1)),
            "iota16": np.tile(np.arange(512, dtype=np.float16), (128, 1)),
            "ident": np.eye(128, dtype=np.float32),
        }
        in_maps.append(m)

    counts = {"NG": NG, "NGTOT": NGTOT, "NBTOT": NBTOT}
    return in_maps, counts


# ----------------------------------------------------------------------------
# Device module
# ----------------------------------------------------------------------------

def _build(C: Cfg, counts, repeat=1):
    ABL = set(os.environ.get("KABL", "").split(","))
    from concourse.tile import add_dep_helper

    NG, NGTOT, NBTOT = counts["NG"], counts["NGTOT"], counts["NBTOT"]
    NL = 4  # GIN layers
    DVE = mybir.EngineType.DVE
    AF = mybir.ActivationFunctionType
    AL = mybir.AluOpType
    BCOLS = C.BG * C.GT * 128 // 16     # idx cols per gather batch
    GCOLS = C.BG * C.GT                 # dstl/xsrc cols per batch
    NIDX = C.BG * C.GT * 128            # idxs per gather

    LOCAL = os.environ.get("KLOCAL") == "1"
    nc = bacc.Bacc("TRN2", target_bir_lowering=False, debug=False,
                   enable_asserts=False, num_swdge_queues=4,
                   num_devices=1 if LOCAL else C.CORES)

    def ein(name, shape, dt=F32):
        return nc.dram_tensor(name, shape, dt, kind="ExternalInput").ap()

    IDX = ein("g_idx", [128, NBTOT * BCOLS], I16)
    DSTL = ein("g_dstl", [128, NGTOT * C.GT])
    XSRC = ein("g_xsrc", [128, NGTOT * C.GT])
    FLUSH = ein("g_flush", [1, NGTOT], I32)
    XOWN = ein("xown", [1, C.NPC_PAD])
    N2S = ein("n2s", [128, C.NT_REAL])
    S2G = ein("s2g", [128, C.S_T])
    SSTART = ein("sstart", [1, 1], I32)
    W1A, B1A = ein("w1a", [1, C.H]), ein("b1a", [C.H, 1])
    W2A, B2A = ein("w2a", [C.H, C.H]), ein("b2a", [C.H, 1])
    CW1, CB1 = ein("cw1", [C.H, 3 * C.H]), ein("cb1", [C.H, 3])
    CW2, CB2 = ein("cw2", [C.H, 3 * C.H]), ein("cb2", [C.H, 3])
    L1W, L1B = ein("l1w", [C.H, C.H]), ein("l1b", [C.H, 1])
    L2W, L2B = ein("l2w", [C.H, C.OUT]), ein("l2b", [C.OUT, 1])
    IOTA = ein("iota", [128, 512])
    IDENT = ein("ident", [128, 128])
    OUTT = nc.dram_tensor("out", [C.NGRAPH, C.OUT], F32, kind="ExternalOutput").ap()

    hbuf = [nc.dram_tensor(f"hbuf{k}", [C.N, C.H], F32, kind="Internal",
                           addr_space="Shared").ap() for k in range(2)]
    agin = [nc.dram_tensor(f"agin{k}", [C.NPC, C.H], F32, kind="Internal").ap()
            for k in range(2)]
    ppin = nc.dram_tensor("ppin", [C.H, C.SPAD], F32, kind="Internal").ap()
    ppout = nc.dram_tensor("ppout", [C.H, C.SPAD], F32, kind="Internal",
                           addr_space="Shared").ap()

    RG = [list(range(C.CORES))]
    AGGW = C.NPC_PAD + C.W

    with tile.TileContext(nc) as tc:
        from concourse import library_config
        nc.gpsimd.load_library(library_config.mlp)
        with (
            tc.tile_pool(name="const", bufs=1) as P0,
            tc.tile_pool(name="stream", bufs=4) as PS,
            tc.tile_pool(name="msgs", bufs=2) as PM,
            tc.tile_pool(name="msgs_b", bufs=2) as PMB,
            tc.tile_pool(name="oh", bufs=4) as PO,
            tc.tile_pool(name="mlp", bufs=2) as PL,
            tc.tile_pool(name="ps_sc", bufs=3, space="PSUM") as PSC,
            tc.tile_pool(name="ps_mlp", bufs=2, space="PSUM") as PSM,
            tc.tile_pool(name="ps_tr", bufs=2, space="PSUM") as PST,
            tc.tile_pool(name="ps_pool", bufs=1, space="PSUM") as PSP,
        ):
            # ---- resident tiles ----
            def load(ap, shape, dt=F32, tag=None):
                t = P0.tile(shape, dt, tag=tag)
                nc.sync.dma_start(t[:], ap)
                return t

            flush_sb = load(FLUSH, [1, NGTOT], I32, tag="c_flush")
            n2s_sb = load(N2S, [128, C.NT_REAL], tag="c_n2s")
            s2g_sb = load(S2G, [128, C.S_T], tag="c_s2g")
            sstart_sb = load(SSTART, [1, 1], I32, tag="c_sstart")
            w1a_sb, b1a_sb = load(W1A, [1, C.H], tag="c_w1a"), load(B1A, [C.H, 1], tag="c_b1a")
            w2a_sb, b2a_sb = load(W2A, [C.H, C.H], tag="c_w2a"), load(B2A, [C.H, 1], tag="c_b2a")
            cw1_sb, cb1_sb = load(CW1, [C.H, 3 * C.H], tag="c_cw1"), load(CB1, [C.H, 3], tag="c_cb1")
            cw2_sb, cb2_sb = load(CW2, [C.H, 3 * C.H], tag="c_cw2"), load(CB2, [C.H, 3], tag="c_cb2")
            l1w_sb, l1b_sb = load(L1W, [C.H, C.H], tag="c_l1w"), load(L1B, [C.H, 1], tag="c_l1b")
            l2w_sb, l2b_sb = load(L2W, [C.H, C.OUT], tag="c_l2w"), load(L2B, [C.OUT, 1], tag="c_l2b")
            iota_sb = load(IOTA, [128, 512], tag="c_iota")
            iota16_sb = load(ein("iota16", [128, 512], F16), [128, 512], F16,
                             tag="c_iota16")
            ident_sb = load(IDENT, [128, 128], tag="c_ident")

            agg = P0.tile([C.H, AGGW], F32, tag="agg")
            hT = P0.tile([C.H, C.NPC_PAD], F32, tag="hT")
            expb = P0.tile([128, C.NT_REAL * C.H], F32, tag="expb")
            pp_sb = P0.tile([C.H, C.SPAD], F32, tag="pp_sb")
            p_sb = P0.tile([C.H, C.SPAD], F32, tag="p_sb")

            regs = [nc.alloc_registers(f"rof{k}", engines=[DVE]) for k in range(4)]
            sreg = nc.alloc_registers("sreg", engines=[DVE])

            ag_inst = None  # last AllGather instruction (DRAM dep anchor)

            for _rep in range(repeat):
              for layer in range(NL):
                  l0 = layer == 0
                  nc.vector.memset(agg[:], 0.0)
                  if l0:
                      w1, b1, w2, b2 = w1a_sb, b1a_sb, w2a_sb, b2a_sb
                  else:
                      r0 = (layer - 1) * C.H
                      li = layer - 1
                      w1 = cw1_sb[:, r0:r0 + C.H]
                      b1 = cb1_sb[:, li:li + 1]
                      w2 = cw2_sb[:, r0:r0 + C.H]
                      b2 = cb2_sb[:, li:li + 1]
                  src_hbuf = hbuf[(layer + 1) % 2] if not l0 else None

                  # ---------- aggregation ----------
                  g_i = 0
                  b_i = 0
                  for cv in range(C.NCH):
                      view = None
                      if not l0:
                          view = src_hbuf[cv * C.CHUNK:(cv + 1) * C.CHUNK, :]
                      for b in range(NG[cv] // C.BG):
                          if l0:
                              xt = PS.tile([128, GCOLS], F32, tag="xsrc_t")
                              nc.sync.dma_start(
                                  xt[:], XSRC[:, g_i * C.GT:g_i * C.GT + GCOLS])
                              mtb = PS.tile([128, GCOLS], BF16, tag="xsrc_b")
                              nc.scalar.activation(mtb[:], xt[:], AF.Copy)
                          else:
                              it = PS.tile([128, BCOLS], I16, tag="idx_t")
                              nc.sync.dma_start(
                                  it[:], IDX[:, b_i * BCOLS:(b_i + 1) * BCOLS])
                              mt = PM.tile([128, GCOLS, C.H], F32, tag="msgs")
                              gi = nc.gpsimd.dma_gather(
                                  mt[:], view, it[:], NIDX, NIDX, C.H,
                                  single_packet=False, queue_num=b_i % 4)
                              if ag_inst is not None:
                                  add_dep_helper(gi.ins, ag_inst.ins,
                                                 reason="gather after allgather")
                              mtb = PMB.tile([128, GCOLS, C.H], BF16, tag="msgs_b")
                              nc.scalar.activation(mtb[:], mt[:], AF.Copy)
                          dt_t = PS.tile([128, GCOLS], F32, tag="dstl_t")
                          nc.sync.dma_start(
                              dt_t[:], DSTL[:, g_i * C.GT:g_i * C.GT + GCOLS])
                          for gg in range(C.BG):
                              ps = PSC.tile([1 if l0 else C.H, C.W], F32, tag="ps_sc")
                              for t in range(C.GT):
                                  tcol = gg * C.GT + t
                                  oh = PO.tile([128, C.W], BF16, tag="oh")
                                  nc.vector.tensor_scalar(
                                      oh[:], iota16_sb[:, :C.W],
                                      dt_t[:, tcol:tcol + 1], None, AL.is_equal)
                                  lhsT = (mtb[:, tcol:tcol + 1] if l0 else
                                          mtb[:, tcol, :])
                                  nc.tensor.matmul(ps[:], lhsT, oh[:],
                                                   start=(t == 0), stop=(t == C.GT - 1))
                              r = regs[g_i % 4]
                              nc.vector.reg_load(r, flush_sb[0:1, g_i:g_i + 1])
                              off = nc.vector.snap(r, donate=False, min_val=0,
                                                   max_val=C.NPC)
                              dyn = agg[0:1, bass.ds(off, C.W)] if l0 else \
                                  agg[:, bass.ds(off, C.W)]
                              nc.vector.tensor_tensor(dyn, ps[:], dyn, AL.add)
                              g_i += 1
                          b_i += 1
                  assert g_i == NGTOT

                  # ---------- update (h + agg -> MLP) ----------
                  for ngp in range(C.NMG):
                      sl = slice(ngp * 512, (ngp + 1) * 512)
                      if l0:
                          xo = PL.tile([1, 512], F32, tag="xo")
                          nc.sync.dma_start(xo[:], XOWN[0:1, sl])
                          hin = PL.tile([1, 512], F32, tag="hin0")
                          nc.vector.tensor_tensor(hin[:], xo[:], agg[0:1, sl], AL.add)
                      else:
                          hin = PL.tile([C.H, 512], F32, tag="hin")
                          nc.vector.tensor_tensor(hin[:], hT[:, sl], agg[:, sl], AL.add)
                      ps1 = PSM.tile([C.H, 512], F32, tag="ps_mlp")
                      nc.tensor.matmul(ps1[:], w1, hin[:], start=True, stop=True)
                      t1 = PL.tile([C.H, 512], F32, tag="t1")
                      nc.scalar.activation(t1[:], ps1[:], AF.Relu, bias=b1)
                      ps2 = PSM.tile([C.H, 512], F32, tag="ps_mlp")
                      nc.tensor.matmul(ps2[:], w2, t1[:], start=True, stop=True)
                      nc.scalar.activation(hT[:, sl], ps2[:], AF.Relu, bias=b2)

                  # ---------- transpose to node-major ----------
                  last = layer == NL - 1
                  if last:
                      ps_s = PSP.tile([C.H, C.SWIN], F32, tag="ps_pool")
                  for j in range(C.NT_REAL):
                      pt = PST.tile([128, C.H], F32, tag="ps_tr")
                      nc.tensor.transpose(pt[:], hT[:, j * 128:(j + 1) * 128],
                                          ident_sb[:C.H, :C.H])
                      nc.scalar.activation(expb[:, j * C.H:(j + 1) * C.H], pt[:],
                                           AF.Copy)
                      if last:
                          ohs = PO.tile([128, C.SWIN], F32, tag="ohs")
                          nc.vector.tensor_scalar(ohs[:], iota_sb[:, :C.SWIN],
                                                  n2s_sb[:, j:j + 1], None,
                                                  AL.is_equal)
                          nc.tensor.matmul(ps_s[:],
                                           expb[:, j * C.H:(j + 1) * C.H],
                                           ohs[:], start=(j == 0),
                                           stop=(j == C.NT_REAL - 1))

                  if not last:
                      # export + AllGather
                      dst = agin[layer % 2]
                      nf = C.NT_REAL - 1
                      d1 = nc.sync.dma_start(
                          dst[0:nf * 128, :].rearrange("(b p) f -> p b f", p=128),
                          expb[:, :nf * C.H].rearrange("p (b f) -> p b f", f=C.H))
                      d2 = nc.sync.dma_start(
                          dst[nf * 128:C.NPC, :],
                          expb[:C.LAST_ROWS, nf * C.H:(nf + 1) * C.H])
                      if LOCAL:
                          ag = nc.sync.dma_start(
                              hbuf[layer % 2][0:C.NPC, :], dst)
                      else:
                          ag = nc.gpsimd.collective_compute(
                              "AllGather", AL.bypass, replica_groups=RG,
                              ins=[dst], outs=[hbuf[layer % 2]])
                      add_dep_helper(ag.ins, d1.ins, reason="ag after export")
                      add_dep_helper(ag.ins, d2.ins, reason="ag after export")
                      ag_inst = ag

            # ---------- subgraph partial sums -> AllReduce ----------
            nc.vector.memset(pp_sb[:], 0.0)
            nc.vector.reg_load(sreg, sstart_sb[0:1, 0:1])
            soff = nc.vector.snap(sreg, donate=True, min_val=0,
                                  max_val=C.SPAD - C.SWIN)
            dynp = pp_sb[:, bass.ds(soff, C.SWIN)]
            nc.vector.tensor_copy(dynp, ps_s[:])
            d3 = nc.sync.dma_start(ppin, pp_sb[:])
            if LOCAL:
                ar = nc.sync.dma_start(ppout, ppin)
            else:
                ar = nc.gpsimd.collective_compute(
                    "AllReduce", AL.add, replica_groups=RG, ins=[ppin], outs=[ppout])
            add_dep_helper(ar.ins, d3.ins, reason="ar after store")
            d4 = nc.sync.dma_start(p_sb[:], ppout)
            add_dep_helper(d4.ins, ar.ins, reason="load after ar")

            # ---------- graph pooling ----------
            ps_g = PSP.tile([C.H, C.NGRAPH], F32, tag="ps_pool")
            for jt in range(C.S_T):
                pt = PST.tile([128, C.H], F32, tag="ps_tr")
                nc.tensor.transpose(pt[:], p_sb[:, jt * 128:(jt + 1) * 128],
                                    ident_sb[:C.H, :C.H])
                sm = PL.tile([128, C.H], F32, tag="smaj")
                nc.scalar.activation(sm[:], pt[:], AF.Copy)
                ohg = PO.tile([128, C.NGRAPH], F32, tag="ohg")
                nc.vector.tensor_scalar(ohg[:], iota_sb[:, :C.NGRAPH],
                                        s2g_sb[:, jt:jt + 1], None, AL.is_equal)
                nc.tensor.matmul(ps_g[:], sm[:], ohg[:], start=(jt == 0),
                                 stop=(jt == C.S_T - 1))

            # ---------- head ----------
            g_sb = PL.tile([C.H, C.NGRAPH], F32, tag="gsb")
            nc.scalar.activation(g_sb[:], ps_g[:], AF.Copy)
            ph1 = PSM.tile([C.H, C.NGRAPH], F32, tag="ps_mlp")
            nc.tensor.matmul(ph1[:], l1w_sb[:], g_sb[:], start=True, stop=True)
            t1h = PL.tile([C.H, C.NGRAPH], F32, tag="t1h")
            nc.scalar.activation(t1h[:], ph1[:], AF.Relu, bias=l1b_sb[:])
            ph2 = PSM.tile([C.OUT, C.NGRAPH], F32, tag="ps_mlp")
            nc.tensor.matmul(ph2[:], l2w_sb[:], t1h[:], start=True, stop=True)
            t2h = PL.tile([C.OUT, C.NGRAPH], F32, tag="t2h")
            nc.scalar.activation(t2h[:], ph2[:], AF.Identity, bias=l2b_sb[:])

            # ---------- log_softmax over classes ----------
            ptz = PST.tile([128, C.OUT], F32, tag="ps_tr")
            nc.tensor.transpose(ptz[:C.NGRAPH, :], t2h[:],
                                ident_sb[:C.OUT, :C.OUT])
            z = PL.tile([C.NGRAPH, C.OUT], F32, tag="z")
            nc.scalar.activation(z[:], ptz[:C.NGRAPH, :], AF.Copy)
            mx = PL.tile([C.NGRAPH, 1], F32, tag="mx")
            nc.vector.tensor_reduce(mx[:], z[:], mybir.AxisListType.X, AL.max)
            zc = PL.tile([C.NGRAPH, C.OUT], F32, tag="zc")
            nc.vector.tensor_scalar(zc[:], z[:], mx[:], None, AL.subtract)
            ex = PL.tile([C.NGRAPH, C.OUT], F32, tag="ex")
            nc.scalar.activation(ex[:], zc[:], AF.Exp)
            sm2 = PL.tile([C.NGRAPH, 1], F32, tag="sm2")
            nc.vector.tensor_reduce(sm2[:], ex[:], mybir.AxisListType.X, AL.add)
            ls = PL.tile([C.NGRAPH, 1], F32, tag="ls")
            nc.scalar.activation(ls[:], sm2[:], AF.Ln)
            res = PL.tile([C.NGRAPH, C.OUT], F32, tag="res")
            nc.vector.tensor_scalar(res[:], zc[:], ls[:], None, AL.subtract)
            nc.sync.dma_start(OUTT, res[:])

    nc.compile()
    return nc


# ----------------------------------------------------------------------------
# Runner
# ----------------------------------------------------------------------------

_CACHE = {}


def _run_sim(nc, in_maps, C: Cfg):
    from concourse.bass_interp import MultiCoreSim
    sim = MultiCoreSim(nc, num_cores=C.CORES, trace=False,
                       require_finite=False, require_nnan=False)
    for c in range(C.CORES):
        for k, v in in_maps[c].items():
            sim.cores[c].tensor(k)[:] = v
    sim.simulate(check_with_hw=False)
    return np.array(sim.cores[0].mem_tensor("out"))


def _run_hw(nc, in_maps, C: Cfg, trace=False, tmpdir=None):
    from concourse.bass_utils import run_bass_kernel_spmd
    res = run_bass_kernel_spmd(nc, in_maps, core_ids=list(range(C.CORES)),
                               trace=trace, tmpdir=tmpdir)
    return res.results[0]["out"], res


def kernel(**inputs):
    C = Cfg()
    in_maps, counts = _prep(inputs, C)
    key = ("full", tuple(counts["NG"]))
    if key not in _CACHE:
        _CACHE[key] = _build(C, counts)
    out, _ = _run_hw(_CACHE[key], in_maps, C)
    return np.asarray(out, np.float32)

